# revision 3
# baseline (speedup 1.0000x reference)
"""Trainium2 Bass kernel for nn_AffineAdapter (Gaussian blur + affine grid_sample).

The reference pipeline (separable 8-tap Gaussian blur -> bilinear grid_sample on
a 25x25 grid, align_corners=True, zero padding) is linear in x and separable per
axis, so each (b, c) image reduces to   out = Ay @ X @ Ax^T   with Ay, Ax of
shape (25, 512) combining blur taps and bilinear weights.  Output sample row p
only reads the 9 input rows [ry(p), ry(p)+9) and output sample col q only the 9
input cols [rx(q), rx(q)+9), so exactly 25*9 = 225 rows x 225 cols of each
512x512 image carry information.  The host gathers that 225x225 block per image
(pure data movement), and the device does the two matmul contractions.

Distribution: pure data parallel over B*C = 128 images -> 16 images per core on
8 NeuronCores.  Per-core layout (all fp16 for TensorE speed; 2e-2 rel-err gate
leaves ~40x headroom over fp16 rounding):

  xs  [2, 128, 16, 225]  gathered rows chunked to 128+97 partitions; per
                         (chunk, partition) line is img-major contiguous so a
                         4-image group DMA is one 1800B run per partition.
  ayt [2, 128, 25]       stage-1 rhs: gathered row k = 9p+j holds Ay[p, ry[p]+j]
                         masked into column p only.
  axt [2, 128, 25]       stage-2 lhsT: same construction for columns/Ax.

  stage 1 (per image, per col-chunk cw, accumulating over row-chunks c):
      psum[w, p] += xs[c][:, img, cw*128:...]^T @ ayt[c]      (X stationary)
  stage 2 (two matmuls over col-chunks, all 16 images in one N=400 stream):
      out[q, (img, p)] = sum_cw axt[cw]^T @ tm[cw]
"""

import sys

if "/opt/trn_rl_repo" not in sys.path:
    sys.path.insert(0, "/opt/trn_rl_repo")

import numpy as np

GRID = 25
K = 7
KH = K // 2          # conv padding = 3
NTAPS = K + 1        # 8 taps (torch arange quirk)
BAND = NTAPS + 1     # 9 rows/cols per output sample
NG = GRID * BAND     # 225 gathered rows (and cols) per image
NP1 = NG - 128       # 97 valid partitions in chunk 1
H = W = 512
B, C = 16, 8
N_CORES = 8
NIMG = (B * C) // N_CORES  # images per core
GI = 4                     # images per DMA group
NGRP = NIMG // GI


def _softplus(v):
    v = np.asarray(v)
    return np.log1p(np.exp(-np.abs(v))) + np.maximum(v, 0.0)


def _axis_weights(lin, g, scale_ax, n_in):
    """(GRID, n_in) float64 weight matrix + per-sample band starts r0 such that
    the support of row p lies in [r0[p], r0[p] + BAND)."""
    nb = n_in - 1  # blurred length (conv with K+1 taps, pad K//2 shrinks by 1)
    coord = ((lin * np.float32(scale_ax) + np.float32(1.0))
             * np.float32(0.5) * np.float32(nb - 1)).astype(np.float32)
    c0 = np.floor(coord)
    w1 = (coord - c0).astype(np.float64)
    w0 = 1.0 - w1
    A = np.zeros((GRID, n_in), np.float64)
    g64 = g.astype(np.float64)
    r0 = np.zeros(GRID, np.int64)
    for p in range(GRID):
        r0[p] = int(min(max(c0[p] - KH, 0), n_in - BAND))
        for a, wgt in ((0, w0[p]), (1, w1[p])):
            cc = float(c0[p]) + a
            if not (0.0 <= cc <= nb - 1):
                continue  # zero padding_mode: out-of-range corner contributes 0
            ci = int(min(max(cc, 0.0), nb - 1))
            # blurred[ci] = sum_i g[i] * x[ci + i - KH]
            for i in range(NTAPS):
                src = ci + i - KH
                if 0 <= src < n_in:
                    A[p, src] += wgt * g64[i]
    return A, r0


def _build_weights(log_sigma, log_scale):
    # scalar chain in fp32 to mirror the reference
    scale = _softplus(np.asarray(log_scale, np.float32)).astype(np.float32)
    s_min = np.float32(scale.min())
    sigma_min = np.float32(0.0) if s_min >= 1.0 else np.float32(0.44) * (
        np.float32(1.0) / s_min - np.float32(1.0))
    sigma = np.float32(np.sqrt(sigma_min ** 2
                               + _softplus(np.asarray(log_sigma, np.float32)) ** 2))
    taps = np.arange(-(KH + 1), KH + 1, dtype=np.float32)
    g = np.exp(-0.5 * (taps / sigma) ** 2)
    g = g / g.sum()

    lin = np.linspace(-1.0, 1.0, GRID).astype(np.float32)
    Ay, ry = _axis_weights(lin, g, scale[1], H)  # rows scaled by scale[1] (y)
    Ax, rx = _axis_weights(lin, g, scale[0], W)  # cols scaled by scale[0] (x)
    return Ay, Ax, ry, rx


def _gather_band(A, r0):
    """(2, 128, GRID) fp16: gathered index k = 9*p + j holds A[p, r0[p]+j],
    masked so it only feeds output sample p."""
    g64 = np.zeros((2 * 128, GRID), np.float64)
    for p in range(GRID):
        sup = np.nonzero(A[p])[0]
        if len(sup) and not (r0[p] <= sup[0] and sup[-1] < r0[p] + BAND):
            raise AssertionError("band does not cover sample support")
        for j in range(BAND):
            g64[BAND * p + j, p] = A[p, int(r0[p]) + j]
    return g64.reshape(2, 128, GRID).astype(np.float16)


_PROGRAM_CACHE = {}


def _build_program():
    import concourse.tile as tile
    from concourse import bacc, mybir

    f32 = mybir.dt.float32
    f16 = mybir.dt.float16

    nc = bacc.Bacc("TRN2", target_bir_lowering=False, debug=False,
                   num_devices=N_CORES)
    xs = nc.dram_tensor("xs", [2, 128, NIMG, NG], f16, kind="ExternalInput")
    ayt = nc.dram_tensor("ayt", [2, 128, GRID], f16, kind="ExternalInput")
    axt = nc.dram_tensor("axt", [2, 128, GRID], f16, kind="ExternalInput")
    out = nc.dram_tensor("out", [GRID, NIMG, GRID], f32, kind="ExternalOutput")

    kchunk = (128, NP1)  # valid partitions (gathered rows/cols) per chunk

    with tile.TileContext(nc) as tc:
        with (
            tc.tile_pool(name="const", bufs=1) as const_pool,
            tc.tile_pool(name="xp", bufs=NGRP) as xpool,
            tc.tile_pool(name="ps1", bufs=NGRP, space="PSUM") as psum1,
            tc.tile_pool(name="ps2", bufs=1, space="PSUM") as psum2,
        ):
            aytile = const_pool.tile([128, 2, GRID], f16)
            nc.sync.dma_start(out=aytile[:], in_=ayt.rearrange("c p n -> p c n"))
            axtile = const_pool.tile([128, 2, GRID], f16)
            nc.scalar.dma_start(out=axtile[:], in_=axt.rearrange("c p n -> p c n"))

            # stage-1 results, keyed [w-part, col-chunk, img, p]
            tm = const_pool.tile([128, 2, NIMG, GRID], f16)

            xt = [[None, None] for _ in range(NGRP)]
            for g in range(NGRP):
                for c in range(2):
                    kc = kchunk[c]
                    t = xpool.tile([128, GI, NG], f16, tag=f"x{c}")
                    eng = nc.sync if c == 0 else nc.scalar
                    eng.dma_start(out=t[:kc],
                                  in_=xs[c, :kc, g * GI:(g + 1) * GI, :])
                    xt[g][c] = t

            for g in range(NGRP):
                ps = psum1.tile([128, 2, GI, GRID], f32)
                for i4 in range(GI):
                    for cw in range(2):
                        m = kchunk[cw]
                        for c in range(2):
                            kc = kchunk[c]
                            nc.tensor.matmul(
                                ps[:m, cw, i4, :],
                                xt[g][c][:kc, i4, cw * 128:cw * 128 + m],
                                aytile[:kc, c, :],
                                start=(c == 0),
                                stop=(c == 1),
                            )
                # drain psum -> tm (cast fp32 -> fp16); split across engines
                nc.vector.tensor_copy(tm[:, 0, g * GI:(g + 1) * GI, :],
                                      ps[:, 0, :, :])
                nc.scalar.copy(tm[:NP1, 1, g * GI:(g + 1) * GI, :],
                               ps[:NP1, 1, :, :])

            # stage 2: all 16 images in one N=400 stream per col-chunk
            po = psum2.tile([GRID, NIMG, GRID], f32)
            for cw in range(2):
                kc = kchunk[cw]
                nc.tensor.matmul(
                    po[:],
                    axtile[:kc, cw, :],
                    tm[:kc, cw, :, :],
                    start=(cw == 0),
                    stop=(cw == 1),
                )
            outst = const_pool.tile([GRID, NIMG, GRID], f32)
            nc.vector.tensor_copy(outst[:], po[:])
            nc.sync.dma_start(out=out[:], in_=outst[:])

    nc.compile()
    return nc


def _get_program():
    if "prog" not in _PROGRAM_CACHE:
        _PROGRAM_CACHE["prog"] = _build_program()
    return _PROGRAM_CACHE["prog"]


def _prepare(log_sigma, log_scale):
    Ay, Ax, ry, rx = _build_weights(log_sigma, log_scale)
    ayt = _gather_band(Ay, ry)
    axt = _gather_band(Ax, rx)
    return ayt, axt, ry, rx


def _pack_x(x, ry, rx):
    """Gather the 225 banded rows x 225 banded cols of each image, split rows
    into 2 partition chunks.  Returns (N_CORES, 2, 128, NIMG, 225) fp16."""
    xf = np.asarray(x, np.float32).reshape(B * C, H, W)
    rows = (np.repeat(np.asarray(ry, np.int64), BAND)
            + np.tile(np.arange(BAND), GRID))        # (225,)
    cols = (np.repeat(np.asarray(rx, np.int64), BAND)
            + np.tile(np.arange(BAND), GRID))        # (225,)
    crop = xf[:, rows][:, :, cols].astype(np.float16)   # (BC, 225, 225)
    pad = np.zeros((B * C, 2 * 128, NG), np.float16)
    pad[:, :NG, :] = crop
    # (BC, 256, 225) -> (cores, img, chunk, part, w) -> (cores, chunk, part, img, w)
    xp = pad.reshape(N_CORES, NIMG, 2, 128, NG).transpose(0, 2, 3, 1, 4)
    return np.ascontiguousarray(xp)


def _make_inmaps(x, log_sigma, log_scale):
    ayt, axt, ry, rx = _prepare(log_sigma, log_scale)
    xp = _pack_x(x, ry, rx)
    return [{"xs": xp[i], "ayt": ayt, "axt": axt} for i in range(N_CORES)]


def _assemble(results):
    out = np.empty((B * C, GRID, GRID), np.float32)
    for i in range(N_CORES):
        # per-core output is (GRID, NIMG, GRID) = (q, img, p)
        out[i * NIMG:(i + 1) * NIMG] = results[i]["out"].transpose(1, 2, 0)
    return out.reshape(B, C, GRID, GRID)


def kernel(x, log_sigma, log_scale):
    from concourse.bass_utils import run_bass_kernel_spmd

    x = np.ascontiguousarray(np.asarray(x, np.float32))
    assert x.shape == (B, C, H, W), x.shape

    nc = _get_program()
    in_maps = _make_inmaps(x, log_sigma, log_scale)
    res = run_bass_kernel_spmd(nc, in_maps, core_ids=list(range(N_CORES)))
    return _assemble(res.results)


# revision 7
# speedup vs baseline: 2.3147x; 2.3147x over previous
"""Trainium2 Bass kernel for nn_AffineAdapter (Gaussian blur + affine grid_sample).

The reference pipeline (separable 8-tap Gaussian blur -> bilinear grid_sample on
a 25x25 grid, align_corners=True, zero padding) is linear in x and separable per
axis, so each (b, c) image reduces to   out = Ay @ X @ Ax^T   with Ay, Ax of
shape (25, 512) combining blur taps and bilinear weights.  Output sample row p
only reads the 9 input rows [ry(p), ry(p)+9) and output sample col q only the 9
input cols [rx(q), rx(q)+9), so exactly 25*9 = 225 rows x 225 cols of each
512x512 image carry information.  The host gathers that 225x225 block per image
(pure data movement), and the device does the two matmul contractions.

Distribution: pure data parallel over B*C = 128 images -> 16 images per core on
8 NeuronCores.  Per-core layout (all fp16 for TensorE speed; 2e-2 rel-err gate
leaves ~40x headroom over fp16 rounding):

  xs  [4, 128, 2, 4, 225]  gathered rows chunked to 128+97(+pad) partitions,
                         grouped 4 images per DMA; each group is one fully
                         contiguous 460KB block (3600B per partition line) so
                         HWDGE descriptor generation stays on the fast path.
  ayt [128, 2, 25]       stage-1 rhs: gathered row k = 9p+j holds Ay[p, ry[p]+j]
                         masked into column p only (pre-transposed on host so
                         the DMA source is contiguous).
  axt [128, 2, 25]       stage-2 lhsT: same construction for columns/Ax.

  stage 1 (per image, per col-chunk cw, accumulating over row-chunks c):
      psum[w, p] += xs[c][:, img, cw*128:...]^T @ ayt[c]      (X stationary)
  stage 2 (two matmuls over col-chunks, all 16 images in one N=400 stream):
      out[q, (img, p)] = sum_cw axt[cw]^T @ tm[cw]
"""

import sys

if "/opt/trn_rl_repo" not in sys.path:
    sys.path.insert(0, "/opt/trn_rl_repo")

import numpy as np

GRID = 25
K = 7
KH = K // 2          # conv padding = 3
NTAPS = K + 1        # 8 taps (torch arange quirk)
BAND = NTAPS + 1     # 9 rows/cols per output sample
NG = GRID * BAND     # 225 gathered rows (and cols) per image
NP1 = NG - 128       # 97 valid partitions in chunk 1
H = W = 512
B, C = 16, 8
N_CORES = 8
NIMG = (B * C) // N_CORES  # images per core
GI = 4                     # images per DMA group
NGRP = NIMG // GI


def _softplus(v):
    v = np.asarray(v)
    return np.log1p(np.exp(-np.abs(v))) + np.maximum(v, 0.0)


def _axis_weights(lin, g, scale_ax, n_in):
    """(GRID, n_in) float64 weight matrix + per-sample band starts r0 such that
    the support of row p lies in [r0[p], r0[p] + BAND)."""
    nb = n_in - 1  # blurred length (conv with K+1 taps, pad K//2 shrinks by 1)
    coord = ((lin * np.float32(scale_ax) + np.float32(1.0))
             * np.float32(0.5) * np.float32(nb - 1)).astype(np.float32)
    c0 = np.floor(coord)
    w1 = (coord - c0).astype(np.float64)
    w0 = 1.0 - w1
    A = np.zeros((GRID, n_in), np.float64)
    g64 = g.astype(np.float64)
    r0 = np.zeros(GRID, np.int64)
    for p in range(GRID):
        r0[p] = int(min(max(c0[p] - KH, 0), n_in - BAND))
        for a, wgt in ((0, w0[p]), (1, w1[p])):
            cc = float(c0[p]) + a
            if not (0.0 <= cc <= nb - 1):
                continue  # zero padding_mode: out-of-range corner contributes 0
            ci = int(min(max(cc, 0.0), nb - 1))
            # blurred[ci] = sum_i g[i] * x[ci + i - KH]
            for i in range(NTAPS):
                src = ci + i - KH
                if 0 <= src < n_in:
                    A[p, src] += wgt * g64[i]
    return A, r0


def _build_weights(log_sigma, log_scale):
    # scalar chain in fp32 to mirror the reference
    scale = _softplus(np.asarray(log_scale, np.float32)).astype(np.float32)
    s_min = np.float32(scale.min())
    sigma_min = np.float32(0.0) if s_min >= 1.0 else np.float32(0.44) * (
        np.float32(1.0) / s_min - np.float32(1.0))
    sigma = np.float32(np.sqrt(sigma_min ** 2
                               + _softplus(np.asarray(log_sigma, np.float32)) ** 2))
    taps = np.arange(-(KH + 1), KH + 1, dtype=np.float32)
    g = np.exp(-0.5 * (taps / sigma) ** 2)
    g = g / g.sum()

    lin = np.linspace(-1.0, 1.0, GRID).astype(np.float32)
    Ay, ry = _axis_weights(lin, g, scale[1], H)  # rows scaled by scale[1] (y)
    Ax, rx = _axis_weights(lin, g, scale[0], W)  # cols scaled by scale[0] (x)
    return Ay, Ax, ry, rx


def _gather_band(A, r0):
    """(128, 2, GRID) fp16: gathered index k = 9*p + j holds A[p, r0[p]+j],
    masked so it only feeds output sample p; partition-major for the DMA."""
    g64 = np.zeros((2 * 128, GRID), np.float64)
    for p in range(GRID):
        sup = np.nonzero(A[p])[0]
        if len(sup) and not (r0[p] <= sup[0] and sup[-1] < r0[p] + BAND):
            raise AssertionError("band does not cover sample support")
        for j in range(BAND):
            g64[BAND * p + j, p] = A[p, int(r0[p]) + j]
    g16 = g64.reshape(2, 128, GRID).astype(np.float16)
    return np.ascontiguousarray(g16.transpose(1, 0, 2))


_PROGRAM_CACHE = {}


def _build_program():
    import concourse.tile as tile
    from concourse import bacc, mybir

    f32 = mybir.dt.float32
    f16 = mybir.dt.float16

    nc = bacc.Bacc("TRN2", target_bir_lowering=False, debug=False,
                   num_devices=N_CORES)
    xs = nc.dram_tensor("xs", [NGRP, 128, 2, GI, NG], f16, kind="ExternalInput")
    ayt = nc.dram_tensor("ayt", [128, 2, GRID], f16, kind="ExternalInput")
    axt = nc.dram_tensor("axt", [128, 2, GRID], f16, kind="ExternalInput")
    out = nc.dram_tensor("out", [GRID, NIMG, GRID], f32, kind="ExternalOutput")

    kchunk = (128, NP1)  # valid partitions (gathered rows/cols) per chunk

    with tile.TileContext(nc) as tc:
        with (
            tc.tile_pool(name="const", bufs=1) as const_pool,
            tc.tile_pool(name="xp", bufs=NGRP) as xpool,
            tc.tile_pool(name="ps1", bufs=NGRP, space="PSUM") as psum1,
            tc.tile_pool(name="ps2", bufs=1, space="PSUM") as psum2,
        ):
            aytile = const_pool.tile([128, 2, GRID], f16)
            nc.sync.dma_start(out=aytile[:], in_=ayt[:])
            axtile = const_pool.tile([128, 2, GRID], f16)
            nc.scalar.dma_start(out=axtile[:], in_=axt[:])

            # stage-1 results, keyed [w-part, col-chunk, img, p]
            tm = const_pool.tile([128, 2, NIMG, GRID], f16)

            xt = []
            for g in range(NGRP):
                t = xpool.tile([128, 2, GI, NG], f16)
                eng = nc.sync if g % 2 == 0 else nc.scalar
                eng.dma_start(out=t[:], in_=xs[g])
                xt.append(t)

            for g in range(NGRP):
                ps = psum1.tile([128, 2, GI, GRID], f32)
                for i4 in range(GI):
                    for cw in range(2):
                        m = kchunk[cw]
                        for c in range(2):
                            kc = kchunk[c]
                            nc.tensor.matmul(
                                ps[:m, cw, i4, :],
                                xt[g][:kc, c, i4, cw * 128:cw * 128 + m],
                                aytile[:kc, c, :],
                                start=(c == 0),
                                stop=(c == 1),
                            )
                # drain psum -> tm (cast fp32 -> fp16)
                nc.vector.tensor_copy(tm[:, 0, g * GI:(g + 1) * GI, :],
                                      ps[:, 0, :, :])
                nc.vector.tensor_copy(tm[:NP1, 1, g * GI:(g + 1) * GI, :],
                                      ps[:NP1, 1, :, :])

            # stage 2: all 16 images in one N=400 stream per col-chunk
            po = psum2.tile([GRID, NIMG, GRID], f32)
            for cw in range(2):
                kc = kchunk[cw]
                nc.tensor.matmul(
                    po[:],
                    axtile[:kc, cw, :],
                    tm[:kc, cw, :, :],
                    start=(cw == 0),
                    stop=(cw == 1),
                )
            outst = const_pool.tile([GRID, NIMG, GRID], f32)
            nc.vector.tensor_copy(outst[:], po[:])
            nc.sync.dma_start(out=out[:], in_=outst[:])

    nc.compile()
    return nc


def _get_program():
    if "prog" not in _PROGRAM_CACHE:
        _PROGRAM_CACHE["prog"] = _build_program()
    return _PROGRAM_CACHE["prog"]


def _prepare(log_sigma, log_scale):
    Ay, Ax, ry, rx = _build_weights(log_sigma, log_scale)
    ayt = _gather_band(Ay, ry)
    axt = _gather_band(Ax, rx)
    return ayt, axt, ry, rx


def _pack_x(x, ry, rx):
    """Gather the 225 banded rows x 225 banded cols of each image, split rows
    into 2 partition chunks of 128 (rows 225.. are zero), group 4 images per
    DMA.  Returns (N_CORES, NGRP, 128, 2, GI, 225) fp16 — each (core, group)
    block is contiguous, 3600B per partition line."""
    xf = np.asarray(x, np.float32).reshape(B * C, H, W)
    rows = (np.repeat(np.asarray(ry, np.int64), BAND)
            + np.tile(np.arange(BAND), GRID))        # (225,)
    cols = (np.repeat(np.asarray(rx, np.int64), BAND)
            + np.tile(np.arange(BAND), GRID))        # (225,)
    crop = xf[:, rows][:, :, cols].astype(np.float16)   # (BC, 225, 225)
    pad = np.zeros((B * C, 2 * 128, NG), np.float16)
    pad[:, :NG, :] = crop
    # (core, g, i, c, p, w) -> (core, g, p, c, i, w)
    xp = pad.reshape(N_CORES, NGRP, GI, 2, 128, NG).transpose(0, 1, 4, 3, 2, 5)
    return np.ascontiguousarray(xp)


def _make_inmaps(x, log_sigma, log_scale):
    ayt, axt, ry, rx = _prepare(log_sigma, log_scale)
    xp = _pack_x(x, ry, rx)
    return [{"xs": xp[i], "ayt": ayt, "axt": axt} for i in range(N_CORES)]


def _assemble(results):
    out = np.empty((B * C, GRID, GRID), np.float32)
    for i in range(N_CORES):
        # per-core output is (GRID, NIMG, GRID) = (q, img, p)
        out[i * NIMG:(i + 1) * NIMG] = results[i]["out"].transpose(1, 2, 0)
    return out.reshape(B, C, GRID, GRID)


def kernel(x, log_sigma, log_scale):
    from concourse.bass_utils import run_bass_kernel_spmd

    x = np.ascontiguousarray(np.asarray(x, np.float32))
    assert x.shape == (B, C, H, W), x.shape

    nc = _get_program()
    in_maps = _make_inmaps(x, log_sigma, log_scale)
    res = run_bass_kernel_spmd(nc, in_maps, core_ids=list(range(N_CORES)))
    return _assemble(res.results)
